# revision 17
# baseline (speedup 1.0000x reference)
"""Trainium2 Bass kernel for nn_Decoder (attention-LSTM decoder).

Data-parallel across 8 NeuronCores: batch 1024 -> 8 shards of 128.

Per step the reference evaluates tanh(W1 @ [d; c; X_t]) over all T-1
encoder positions. We split W1: the X part (A = W1x@X + b1) is precomputed
once; each step adds only u = [W1d W1c] @ [d; c] broadcast over t.
Softmax runs transposed ([t, b], unnormalized, no max-sub — scores are
bounded by sum|W2| ~ 10) and `context` is never materialized: its two
linear consumers use precomputed XF = X@fc_W[:M] and XFF = X@fcf_W[P:]
reduced against unnormalized beta by tiny ones-matmuls; 1/sum(exp) is
folded into scalar row ops.

Layouts (per core):
  A_T  [128 h', (ht:2, t:TT, b:128)] fp16
  X_T2 [TT t, (b:128, m:256)] fp16      e/scores chunks [h', (ht, tc, b)]
  state dT2/cT [128 p', (pt:2, b:128)] f32 ; dT2 = 2*d (0.5 folded into
  consumer weights); sigma(x) = 0.5*tanh(0.5x)+0.5 with the 0.5-prescale
  folded into the gate weights so ONE plain tanh covers all 4 gates.
"""

import os
import numpy as np
from contextlib import ExitStack

B, T, M, P = 1024, 128, 256, 256
Tm1 = T - 1
NCORES = 8
BC = B // NCORES  # 128

_INPUT_SPECS = [
    ("X_encoded", (BC, Tm1, M)),
    ("y_prev", (BC, Tm1)),
    ("attn_W1", (M, 2 * P + M)),
    ("attn_b1", (M,)),
    ("attn_W2", (1, M)),
    ("attn_b2", (1,)),
    ("W_ih", (4 * P, 1)),
    ("W_hh", (4 * P, P)),
    ("b_ih", (4 * P,)),
    ("b_hh", (4 * P,)),
    ("fc_W", (1, M + 1)),
    ("fc_b", (1,)),
    ("fcf_W", (1, P + M)),
    ("fcf_b", (1,)),
]


def build(t_steps=Tm1, t_chunk=24):
    import concourse.bass as bass
    import concourse.tile as tile
    from concourse import bacc, mybir
    from concourse.masks import make_identity

    ts = bass.ts
    f32 = mybir.dt.float32
    f16 = mybir.dt.float16
    AF = mybir.ActivationFunctionType
    ALU = mybir.AluOpType

    nc = bacc.Bacc("TRN2", target_bir_lowering=False, debug=False)

    dram = {}
    for name, shape in _INPUT_SPECS:
        dram[name] = nc.dram_tensor(name, list(shape), f32, kind="ExternalInput").ap()
    out_dram = nc.dram_tensor("out", [BC, 1], f32, kind="ExternalOutput").ap()

    TT = t_steps
    chunks = []
    t0 = 0
    while t0 < TT:
        chunks.append((t0, min(t_chunk, TT - t0)))
        t0 += t_chunk

    with tile.TileContext(nc) as tc, ExitStack() as ctx:
        # ---------------- pools ----------------
        const = ctx.enter_context(tc.tile_pool(name="const", bufs=1))
        big = ctx.enter_context(tc.tile_pool(name="big", bufs=1))
        small2 = ctx.enter_context(tc.tile_pool(name="small2", bufs=2))
        chunkp = ctx.enter_context(tc.tile_pool(name="chunkp", bufs=2))
        state = ctx.enter_context(tc.tile_pool(name="state", bufs=2))
        _ph = {}  # holder for the currently-active PSUM pool


        # ---------------- constants ----------------
        ident16 = const.tile([128, 128], f16)
        make_identity(nc, ident16[:])
        ident32 = const.tile([128, 128], f32)
        make_identity(nc, ident32[:])
        ones_row = const.tile([1, 128], f16)  # A-mm bias rhs
        nc.gpsimd.memset(ones_row[:], 1.0)
        onesT = const.tile([TT, 1], f16)  # partition-sum lhsT
        nc.gpsimd.memset(onesT[:], 1.0)

        def t_copy(dst, src, scale=None):
            if scale is None:
                nc.vector.tensor_copy(dst, src)
            else:
                nc.vector.tensor_scalar_mul(dst, src, float(scale))

        def pe_transpose(dst_sbuf, src_sbuf, ident, scale=None):
            pt = _ph["p"].tile(
                [src_sbuf.shape[-1], src_sbuf.shape[0]], src_sbuf.dtype, tag="ptr"
            )
            nc.tensor.transpose(pt[:], src_sbuf, ident[:])
            t_copy(dst_sbuf, pt[:], scale)

        # col tile from a DRAM row: [n] f32 -> [n<=128, 1] fp16 (* scale)
        def col16(dst, src_row_ap, scale=None):
            n = dst.shape[0]
            cst = stage.tile([128, 1], f32, tag="colst")
            nc.sync.dma_start(cst[0:n, :], src_row_ap.unsqueeze(1))
            t_copy(dst, cst[0:n, :], scale)

        # ---------------- Phase 0a: weights ----------------
        ctx0 = ExitStack()
        stage = ctx0.enter_context(tc.tile_pool(name="stage", bufs=2))
        psum0 = ctx0.enter_context(tc.tile_pool(name="psum0", bufs=2, space="PSUM"))
        _ph["p"] = psum0
        # attn_W1 [256, 768] -> fp16 [2][128, 768]
        w1_16 = []
        for ht in range(2):
            wst = stage.tile([128, 768], f32, tag="wstage", bufs=1)
            nc.sync.dma_start(wst[:], dram["attn_W1"][ts(ht, 128), :])
            w16 = stage.tile([128, 768], f16, tag="wstage16", bufs=2)
            t_copy(w16[:], wst[:])
            w1_16.append(w16)

        # W1dcT [4 kt][128 p', 256 h]; d-part (kt<2) x0.5
        w1dcT = [const.tile([128, 256], f16, name=f"w1dcT{i}") for i in range(4)]
        for kt in range(4):
            for ht in range(2):
                pe_transpose(
                    w1dcT[kt][:, ts(ht, 128)],
                    w1_16[ht][:, ts(kt, 128)],
                    ident16,
                    scale=0.5 if kt < 2 else None,
                )
        # W1xT [2 kt][128 m', 256 h]
        w1xT = [const.tile([128, 256], f16, name=f"w1xT{i}") for i in range(2)]
        for kt in range(2):
            for ht in range(2):
                pe_transpose(
                    w1xT[kt][:, ts(ht, 128)],
                    w1_16[ht][:, 512 + kt * 128 : 512 + (kt + 1) * 128],
                    ident16,
                )

        # gate tile order: [i0 i1 f0 f1 g0 g1 o0 o1]; sigma gates get x0.5
        def gate_scale(gt):
            return 1.0 if gt in (4, 5) else 0.5

        # W_hhT [2 kt][128 p, 1024 g] ; x(0.5 d-fold) x(sigma prescale)
        whhT = [const.tile([128, 1024], f16, name=f"whhT{i}") for i in range(2)]
        for gt in range(8):
            wst = stage.tile([128, 256], f32, tag="wstage2", bufs=1)
            nc.sync.dma_start(wst[:], dram["W_hh"][ts(gt, 128), :])
            w16 = stage.tile([128, 256], f16, tag="wstage2b")
            t_copy(w16[:], wst[:])
            for kt in range(2):
                pe_transpose(
                    whhT[kt][:, ts(gt, 128)],
                    w16[:, ts(kt, 128)],
                    ident16,
                    scale=0.5 * gate_scale(gt),
                )

        # ktile3 [2, 1024] fp16: row0 = W_ih^T * sig, row1 = (b_ih+b_hh) * sig
        ktile3 = const.tile([2, 1024], f16)
        for gt in range(8):
            wst = stage.tile([128, 1], f32, tag="wih")
            nc.sync.dma_start(wst[:], dram["W_ih"][ts(gt, 128), :])
            w16 = stage.tile([128, 1], f16, tag="wih16")
            t_copy(w16[:], wst[:], scale=gate_scale(gt))
            pe_transpose(ktile3[0:1, ts(gt, 128)], w16[:], ident16)
        for gt in range(8):
            bst0 = stage.tile([1, 128], f32, tag="bg0")
            bst1 = stage.tile([1, 128], f32, tag="bg1")
            nc.sync.dma_start(bst0[:], dram["b_ih"][ts(gt, 128)].unsqueeze(0))
            nc.sync.dma_start(bst1[:], dram["b_hh"][ts(gt, 128)].unsqueeze(0))
            nc.vector.tensor_add(bst0[:], bst0[:], bst1[:])
            bsum16 = stage.tile([1, 128], f16, tag="bg3")
            sc = gate_scale(gt)
            t_copy(bsum16[:], bst0[:], scale=None if sc == 1.0 else sc)
            nc.sync.dma_start(ktile3[1:2, ts(gt, 128)], bsum16[:])

        # small col weights (fp16 [*, 1])
        w2col = [const.tile([128, 1], f16, name=f"w2col{i}") for i in range(2)]
        fcWc = [const.tile([128, 1], f16, name=f"fcWc{i}") for i in range(2)]
        fcfCc = [const.tile([128, 1], f16, name=f"fcfCc{i}") for i in range(2)]
        fcfDc = [const.tile([128, 1], f16, name=f"fcfDc{i}") for i in range(2)]
        for kt in range(2):
            col16(w2col[kt][:], dram["attn_W2"][0, ts(kt, 128)])
            col16(fcWc[kt][:], dram["fc_W"][0, ts(kt, 128)])
            col16(fcfCc[kt][:], dram["fcf_W"][0, P + kt * 128 : P + (kt + 1) * 128])
            col16(fcfDc[kt][:], dram["fcf_W"][0, ts(kt, 128)], scale=0.5)

        # b1 row [1, 256] fp16
        bstb = stage.tile([1, 256], f32, tag="b1")
        nc.sync.dma_start(bstb[:], dram["attn_b1"].unsqueeze(0))
        b1row = const.tile([1, 256], f16)
        t_copy(b1row[:], bstb[:])

        # scalar aps [1,1] f32
        fcWy_ap = const.tile([1, 1], f32)
        nc.sync.dma_start(fcWy_ap[:], dram["fc_W"][0:1, M : M + 1])
        fcb_ap = const.tile([1, 1], f32)
        nc.sync.dma_start(fcb_ap[:], dram["fc_b"].unsqueeze(0))
        fcfb_ap = const.tile([1, 1], f32)
        nc.sync.dma_start(fcfb_ap[:], dram["fcf_b"].unsqueeze(0))

        # y_prevT [TT, 128] f32 (via PE transpose)
        ypst = stage.tile([128, TT], f32, tag="ypst")
        nc.sync.dma_start(ypst[:], dram["y_prev"][:, 0:TT])
        y_prevT = const.tile([TT, 128], f32)
        pe_transpose(y_prevT[:], ypst[:], ident32)

        # ---------------- Phase 0b: X -> A_T, X_T2, XF, XFF ----------------
        A_T = big.tile([128, 2, TT, 128], f16)
        X_T2 = big.tile([TT, 128, 256], f16)
        XF = const.tile([TT, 128], f16)
        XFF = const.tile([TT, 128], f16)

        lt = 0
        while lt < TT:
            lsz = min(1, TT - lt)
            st32 = stage.tile([128, 1, 256], f32, tag="xstage")
            nc.sync.dma_start(st32[:, 0:lsz, :], dram["X_encoded"][:, lt : lt + lsz, :])
            st16 = stage.tile([128, 1, 256], f16, tag="xstage16")
            t_copy(st16[:, 0:lsz, :], st32[:, 0:lsz, :])
            for tr in range(lsz):
                nc.sync.dma_start(
                    X_T2[lt + tr : lt + tr + 1, :, :], st16[:, tr, :]
                )
            for tr in range(lsz):
                t_ = lt + tr
                xtT = stage.tile([128, 2, 128], f16, tag="xtT")
                for mt in range(2):
                    pe_transpose(xtT[:, mt, :], st16[:, tr, ts(mt, 128)], ident16)
                pA = psum0.tile([128, 2, 128], f32, tag="pA")
                for mt in range(2):
                    for kt in range(2):
                        nc.tensor.matmul(
                            pA[:, mt, :],
                            w1xT[kt][:, ts(mt, 128)],
                            xtT[:, kt, :],
                            start=(kt == 0),
                            stop=False,
                        )
                    nc.tensor.matmul(
                        pA[:, mt, :],
                        b1row[0:1, ts(mt, 128)],
                        ones_row[:],
                        start=False,
                        stop=True,
                    )
                    t_copy(A_T[:, mt, t_, :], pA[:, mt, :])
                # XF/XFF rows
                pxf_a = psum0.tile([1, 128], f32, tag="row")
                pxf_b = psum0.tile([1, 128], f32, tag="row")
                for kt in range(2):
                    nc.tensor.matmul(
                        pxf_a[:], fcWc[kt][:], xtT[:, kt, :],
                        start=(kt == 0), stop=(kt == 1),
                    )
                for kt in range(2):
                    nc.tensor.matmul(
                        pxf_b[:], fcfCc[kt][:], xtT[:, kt, :],
                        start=(kt == 0), stop=(kt == 1),
                    )
                xfr = stage.tile([1, 128], f16, tag="xfr")
                t_copy(xfr[:], pxf_a[:])
                nc.sync.dma_start(XF[t_ : t_ + 1, :], xfr[:])
                xffr = stage.tile([1, 128], f16, tag="xffr")
                t_copy(xffr[:], pxf_b[:])
                nc.sync.dma_start(XFF[t_ : t_ + 1, :], xffr[:])
            lt += lsz

        ctx0.close()
        psumS = ctx.enter_context(tc.tile_pool(name="psumS", bufs=1, space="PSUM"))
        _ph["p"] = psumS

        # ---------------- state init ----------------
        dT2 = state.tile([128, 2, 128], f32, tag="dT2")
        cT = state.tile([128, 2, 128], f32, tag="cT")
        nc.vector.memset(dT2[:], 0.0)
        nc.vector.memset(cT[:], 0.0)

        yo_tile = const.tile([2, 128], f16)
        nc.sync.dma_start(yo_tile[1:2, :], ones_row[:])

        # persistent ping-pong score psum tiles (memset once: the extraction
        # copy reads all 128 partitions, matmuls write only rows 0/32/64/96)
        psc0 = psumS.tile([128, 4, 128], f32, tag="psc0", bufs=1)
        psc1 = psumS.tile([128, 4, 128], f32, tag="psc1", bufs=1)
        nc.vector.memset(psc0[:], 0.0)
        nc.vector.memset(psc1[:], 0.0)
        psc_tiles = [psc0, psc1]

        rsum_last = None
        y2u_psum = None

        # ---------------- scan ----------------
        for s in range(TT):
            # fp16 state copy [p', (d0 d1 c0 c1), b]
            dc16 = small2.tile([128, 4, 128], f16, tag="dc16")
            t_copy(dc16[:, 0:2, :], dT2[:])
            t_copy(dc16[:, 2:4, :], cT[:])

            # u^T[h, b] (PE)
            pu = psumS.tile([128, 2, 128], f32, tag="pu", bufs=1)
            for mt in range(2):
                for kt in range(4):
                    nc.tensor.matmul(
                        pu[:, mt, :],
                        w1dcT[kt][:, ts(mt, 128)],
                        dc16[:, kt, :],
                        start=(kt == 0),
                        stop=(kt == 3),
                    )
            u16 = small2.tile([128, 2, 128], f16, tag="u16")
            t_copy(u16[:], pu[:])

            # attention sweep: pre-add (DVE) -> tanh (ACT) -> scores (PE)
            scoresT = small2.tile([128, 128], f32, tag="scoresT")
            nci = 0  # global n-chunk counter (4t per n-chunk)
            psc = None
            for (c0, csz) in chunks:
                ch = chunkp.tile([128, 2, t_chunk, 128], f16, tag="chunk")
                cview = ch[:, :, 0:csz, :]
                nc.vector.tensor_add(
                    cview,
                    A_T[:, :, c0 : c0 + csz, :],
                    u16[:].unsqueeze(2).broadcast_to([128, 2, csz, 128]),
                )
                nc.scalar.activation(cview, cview, AF.Tanh)
                n0 = 0
                while n0 < csz:
                    nsz = min(4, csz - n0)
                    j = nci % 4
                    if j == 0:
                        psc = psc_tiles[(nci // 4) % 2]
                        jt0 = c0 + n0  # first t of this psum group
                    if nsz < 4:
                        nc.vector.memset(psc[32 * j : 32 * j + 1, nsz:4, :], 0.0)
                    for kt in range(2):
                        nc.tensor.matmul(
                            psc[32 * j : 32 * j + 1, 0:nsz, :],
                            w2col[kt][:],
                            ch[:, kt, n0 : n0 + nsz, :],
                            start=(kt == 0),
                            stop=(kt == 1),
                            tile_position=(0, 32 * j),
                        )
                    last = (c0 + n0 + nsz) >= TT
                    if j == 3 or last:
                        # extract psum group rows -> sbuf, DMA-pack to scoresT
                        if last and j < 3:
                            for jj in range(j + 1, 4):
                                nc.vector.memset(
                                    psc[32 * jj : 32 * jj + 1, :, :], 0.0
                                )
                        ssb = small2.tile([128, 4, 128], f32, tag="ssb", bufs=1)
                        t_copy(ssb[:], psc[:])
                        nc.sync.dma_start(
                            scoresT[jt0 : jt0 + 16, :],
                            ssb[0::32, :, :],
                        )
                    n0 += nsz
                    nci += 1

            # exp (unnormalized softmax), fp16
            expT = small2.tile([TT, 128], f16, tag="expT")
            nc.scalar.activation(expT[:], scoresT[0:TT, :], AF.Exp)

            # exf = expT * XF (DVE fp16)
            exf = small2.tile([TT, 128], f16, tag="exf")
            nc.vector.tensor_mul(exf[:], expT[:], XF[:])

            # sumexp + ytu via ones-matmul (PE)
            pr_a = psumS.tile([1, 128], f32, tag="row", bufs=2)
            pr_b = psumS.tile([1, 128], f32, tag="row", bufs=2)
            nc.tensor.matmul(pr_a[:], onesT[:], expT[:], start=True, stop=True)
            nc.tensor.matmul(pr_b[:], onesT[:], exf[:], start=True, stop=True)

            # row ops (DVE, single partition)
            rsum = small2.tile([1, 128], f32, tag="rsum")
            nc.vector.reciprocal(rsum[:], pr_a[:])
            ytr = small2.tile([1, 128], f32, tag="ytr")
            nc.vector.tensor_mul(ytr[:], pr_b[:], rsum[:])
            yp_row = small2.tile([1, 128], f32, tag="yprow")
            nc.sync.dma_start(yp_row[:], y_prevT[s : s + 1, :])
            yt2 = small2.tile([1, 128], f32, tag="yt2")
            nc.vector.scalar_tensor_tensor(
                yt2[:], yp_row[:], fcWy_ap[0:1, 0:1], ytr[:],
                ALU.mult, ALU.add,
            )
            nc.vector.tensor_scalar(
                yo_tile[0:1, :], yt2[:], fcb_ap[0:1, 0:1], None, ALU.add
            )

            # gates (PE): [128, 8, 128] psum, tile order i i f f g g o o
            pg = psumS.tile([128, 8, 128], f32, tag="pg", bufs=1)
            for gt in range(8):
                for kt in range(2):
                    nc.tensor.matmul(
                        pg[:, gt, :],
                        whhT[kt][:, ts(gt, 128)],
                        dc16[:, kt, :],
                        start=(kt == 0),
                        stop=False,
                    )
                nc.tensor.matmul(
                    pg[:, gt, :],
                    ktile3[:, ts(gt, 128)],
                    yo_tile[:],
                    start=False,
                    stop=True,
                )
            # ONE tanh for all gates (sigma prescale folded into weights)
            sig = small2.tile([128, 8, 128], f32, tag="sig", bufs=1)
            nc.scalar.activation(sig[:], pg[:], AF.Tanh)

            # LSTM update (DVE):
            # c = 0.5*((tf+1)*c + (ti+1)*tg);  dT2 = (to+1)*tanh(c)
            tmp1 = small2.tile([128, 2, 128], f32, tag="tmp1", bufs=1)
            nc.vector.scalar_tensor_tensor(
                tmp1[:], sig[:, 2:4, :], 1.0, cT[:], ALU.add, ALU.mult
            )
            tmp2 = small2.tile([128, 2, 128], f32, tag="tmp2", bufs=1)
            nc.vector.scalar_tensor_tensor(
                tmp2[:], sig[:, 0:2, :], 1.0, sig[:, 4:6, :], ALU.add, ALU.mult
            )
            cT_new = state.tile([128, 2, 128], f32, tag="cT")
            nc.vector.tensor_add(cT_new[:], tmp1[:], tmp2[:])
            nc.vector.tensor_scalar_mul(cT_new[:], cT_new[:], 0.5)
            tanh_c = small2.tile([128, 2, 128], f32, tag="tanhc", bufs=1)
            nc.scalar.activation(tanh_c[:], cT_new[:], AF.Tanh)
            dT2_new = state.tile([128, 2, 128], f32, tag="dT2")
            nc.vector.scalar_tensor_tensor(
                dT2_new[:], sig[:, 6:8, :], 1.0, tanh_c[:], ALU.add, ALU.mult
            )
            dT2, cT = dT2_new, cT_new

            if s == TT - 1:
                exff = small2.tile([TT, 128], f16, tag="exff")
                nc.vector.tensor_mul(exff[:], expT[:], XFF[:])
                y2u_psum = psumS.tile([1, 128], f32, tag="y2u", bufs=1)
                nc.tensor.matmul(
                    y2u_psum[:], onesT[:], exff[:], start=True, stop=True
                )
                rsum_last = rsum

        # ---------------- final projection ----------------
        d16f = small2.tile([128, 2, 128], f16, tag="d16f", bufs=1)
        t_copy(d16f[:], dT2[:])
        pf = psumS.tile([1, 128], f32, tag="row", bufs=2)
        for kt in range(2):
            nc.tensor.matmul(
                pf[:], fcfDc[kt][:], d16f[:, kt, :],
                start=(kt == 0), stop=(kt == 1),
            )
        yrow = small2.tile([1, 128], f32, tag="yrow")
        nc.vector.tensor_mul(yrow[:], y2u_psum[:], rsum_last[:])
        nc.vector.tensor_add(yrow[:], yrow[:], pf[:])
        nc.vector.tensor_scalar(
            yrow[:], yrow[:], fcfb_ap[0:1, 0:1], None, ALU.add
        )
        nc.sync.dma_start(out_dram.rearrange("b o -> o b"), yrow[:])

    nc.compile()
    return nc


_NC_CACHE = {}


def _get_nc(t_steps=Tm1):
    if t_steps not in _NC_CACHE:
        _NC_CACHE[t_steps] = build(t_steps=t_steps)
    return _NC_CACHE[t_steps]


def kernel(**inputs):
    from concourse.bass_utils import run_bass_kernel_spmd

    nc = _get_nc()
    full = {k: np.ascontiguousarray(np.asarray(v), dtype=np.float32)
            for k, v in inputs.items()}
    in_maps = []
    for i in range(NCORES):
        sl = slice(i * BC, (i + 1) * BC)
        m = dict(full)
        m["X_encoded"] = full["X_encoded"][sl]
        m["y_prev"] = full["y_prev"][sl]
        in_maps.append(m)
    trace = bool(int(os.environ.get("KERNEL_TRACE", "0")))
    try:
        res = run_bass_kernel_spmd(nc, in_maps, list(range(NCORES)), trace=trace)
    except ModuleNotFoundError:
        res = run_bass_kernel_spmd(nc, in_maps, list(range(NCORES)), trace=False)
    kernel.last_results = res
    return np.concatenate([r["out"] for r in res.results], axis=0)
